# revision 1
# baseline (speedup 1.0000x reference)
"""Trainium2 Bass kernel for nn_CopyMechanism.

Math (per batch b, one NeuronCore per batch):
  out[g,c] = softmax_c(mask ? (score_h[g]+score_c[c]) : -inf)
             * sigmoid(gate_h[g]+gate_c[c]+b0)

softmax_c of (score_h[g]+score_c[c]) == softmax_c(score_c) (score_h constant
along c), so copy_probs is g-independent and w_attn[:H] drops out.
encoder_output is unused by the reference. Scores are O(1), so exp needs no
max subtraction; masking is additive (sc - 1e5 -> exp underflows to exact 0).

Structure (engines pipelined under the ctx DMA stream):
  ctx streams in as 8 chunks of [128,4,1024], cast f32->bf16 in the SWDGE
  DMA (bf16 PE path: fp32 matmul runs LOW_HIGH at ~4x the cycles). Three
  chunk buffers; chunk j+3's DMA is emitted right after chunk j's
  transposes so completions stay progressive and the gpsimd FIFO never
  head-blocks. Per chunk: 32 bf16 PE transposes -> bf16 PSUM, copies to
  SBUF (split vector/scalar), 8 bf16 dot matmuls (weight pair [h,2]
  stationary) -> dots [2,512] f32 (gc row 0, sc row 1); gc broadcast
  (GPSIMD) + 4 sigmoids with per-partition bias gh -> bf16 sig tiles; sc
  DMA-hopped to partition 0 with the mask row DMA-accumulated onto it
  (SWDGE accum), one Exp activation (bf16 out) with fused f32 partial-Z
  accum, one bf16 e broadcast; q[gi] = sig * e_b (bf16 DVE). Post-Z tail:
  Z reduce, 1/Z, per-gi out_t = q * rZ (DVE, f32 out) and 4x [128,4096]
  2MB contiguous DMAs out.
"""
import sys

if "/opt/trn_rl_repo" not in sys.path:
    sys.path.insert(0, "/opt/trn_rl_repo")

import numpy as np
from contextlib import ExitStack

B, G, C, H = 8, 512, 4096, 1024
N_CORES = 8
P = 128
NCT = C // P          # 32 c-tiles of 128
NGT = G // P          # 4 g-tiles of 128
CJ = C // 512         # 8 c-chunks of 512
JH = H // P           # 8 h-blocks of 128

_cache = {}


def _build():
    import concourse.bass as bass
    import concourse.tile as tile
    from concourse import bacc, mybir
    from concourse.masks import make_identity

    f32 = mybir.dt.float32
    bf16 = mybir.dt.bfloat16
    ts = bass.ts

    nc = bacc.Bacc("TRN2", target_bir_lowering=False, debug=False,
                   num_devices=N_CORES)
    hid = nc.dram_tensor("hid", [G, H], f32, kind="ExternalInput").ap()
    ctx_d = nc.dram_tensor("ctx", [C, H], f32, kind="ExternalInput").ap()
    # additive mask row: 0.0 where mask==1 else -1e5
    madd_d = nc.dram_tensor("madd", [1, C], f32, kind="ExternalInput").ap()
    w_d = nc.dram_tensor("w", [3, H], f32, kind="ExternalInput").ap()  # wg_c, wa_c, wg_h
    bg_d = nc.dram_tensor("bg", [1, 1], f32, kind="ExternalInput").ap()
    out_d = nc.dram_tensor("out", [G, C], f32, kind="ExternalOutput").ap()

    with tile.TileContext(nc) as tc:
        with ExitStack() as ctx:
            singles = ctx.enter_context(tc.tile_pool(name="singles", bufs=1))
            hidp = ctx.enter_context(tc.tile_pool(name="hidp", bufs=1))
            ctxp = ctx.enter_context(tc.tile_pool(name="ctxp", bufs=3))
            ctp = ctx.enter_context(tc.tile_pool(name="ctp", bufs=2))
            junkp = ctx.enter_context(tc.tile_pool(name="junkp", bufs=1))
            smp = ctx.enter_context(tc.tile_pool(name="smp", bufs=1))
            qp = ctx.enter_context(tc.tile_pool(name="qp", bufs=1))
            outp = ctx.enter_context(tc.tile_pool(name="outp", bufs=2))
            bp = ctx.enter_context(tc.tile_pool(name="bp", bufs=2))
            sigp = ctx.enter_context(tc.tile_pool(name="sigp", bufs=2))
            rowp = ctx.enter_context(tc.tile_pool(name="rowp", bufs=2))
            tp_ps = ctx.enter_context(
                tc.tile_pool(name="tp_ps", bufs=2, space="PSUM"))
            dt_ps = ctx.enter_context(
                tc.tile_pool(name="dt_ps", bufs=2, space="PSUM"))
            z_ps_p = ctx.enter_context(
                tc.tile_pool(name="z_ps_p", bufs=2, space="PSUM"))

            # ---- ctx chunk DMAs (SWDGE f32->bf16 cast) ----
            ctx4s = []

            def emit_ctx_dma(j, nsub=1):
                ctx4 = ctxp.tile([P, 4, H], bf16, tag="ctx4")
                w = 4 // nsub
                for h2 in range(nsub):
                    nc.gpsimd.dma_start(
                        out=ctx4[:, h2 * w:(h2 + 1) * w, :],
                        in_=ctx_d[j * 512 + h2 * w * P:
                                  j * 512 + (h2 + 1) * w * P, :].rearrange(
                            "(i p) h -> p i h", p=P))
                ctx4s.append(ctx4)

            emit_ctx_dma(0, nsub=2)
            emit_ctx_dma(1, nsub=2)

            # small inputs next on the gpsimd queue
            wpair = singles.tile([2, H], f32)
            nc.gpsimd.dma_start(out=wpair, in_=w_d[0:2, :])
            ident_b = singles.tile([P, P], bf16)
            make_identity(nc, ident_b)
            ident_f = singles.tile([2, 2], f32)
            make_identity(nc, ident_f)

            emit_ctx_dma(2)

            # hid on the HWDGE (sync) queue in f32
            hid4 = hidp.tile([P, NGT, H], f32)
            nc.sync.dma_start(out=hid4,
                              in_=hid.rearrange("(gi p) h -> p gi h", p=P))

            # ---- small constants ----
            whb = singles.tile([P, H], f32)  # wg_h broadcast to partitions
            w_gh = w_d[2:3, :]
            nc.gpsimd.dma_start(
                out=whb,
                in_=bass.AP(tensor=w_gh.tensor, offset=w_gh.offset,
                            ap=[[0, P], [1, H]]))
            bg_b = singles.tile([P, 1], f32)
            nc.gpsimd.dma_start(
                out=bg_b,
                in_=bass.AP(tensor=bg_d.tensor, offset=bg_d.offset,
                            ap=[[0, P], [1, 1]]))

            # w2[h, 2*jh + s] = w[s, jh*128 + h] for s in {0: wg_c, 1: wa_c}
            # (gc lands on dots partition 0 so GPSIMD can broadcast directly)
            w2_ps = z_ps_p.tile([P, 2 * JH], f32, tag="zps")
            for jh in range(JH):
                nc.tensor.transpose(w2_ps[:, jh * 2:jh * 2 + 2],
                                    wpair[:, ts(jh, P)], ident_f)
            w2 = singles.tile([P, 2 * JH], bf16)
            nc.scalar.copy(w2, w2_ps)

            # ---- gh = hid @ wg_h + b_gate  (column layout [128, NGT]) ----
            ghp = smp.tile([P, NGT], f32)
            for gi in range(NGT):
                junk = junkp.tile([P, H], f32, tag="junk")
                nc.vector.tensor_mul(junk, hid4[:, gi, :], whb)
                nc.vector.reduce_sum(ghp[:, gi:gi + 1], junk,
                                     axis=mybir.AxisListType.X)
            gh = smp.tile([P, NGT], f32)
            nc.vector.tensor_scalar(out=gh, in0=ghp, scalar1=bg_b[:, 0:1],
                                    scalar2=None, op0=mybir.AluOpType.add)

            # ---- persistent tiles ----
            z_row = smp.tile([1, CJ], f32)
            q = [qp.tile([P, C], bf16, tag=f"q{gi}", name=f"q{gi}")
                 for gi in range(NGT)]

            # ---- per-chunk pipeline ----
            for j in range(CJ):
                ctx4 = ctx4s[j]
                # 32 bf16 transposes -> 4 PSUM tiles of [P, 1024] bf16,
                # tile t holding h-blocks 2t,2t+1 x 4 c-tiles as [hh,i,128]
                ctxT = ctp.tile([P, JH, 512], bf16, tag="ctxT")
                for t in range(4):
                    tp = tp_ps.tile([P, 1024], bf16, tag="tps")
                    for hh in range(2):
                        jh = t * 2 + hh
                        for i in range(4):
                            nc.tensor.transpose(
                                tp[:, hh * 512 + i * P:hh * 512 + (i + 1) * P],
                                ctx4[:, i, ts(jh, P)], ident_b)
                    dst = ctxT[:, t * 2:(t + 1) * 2, :].rearrange(
                        "p a b -> p (a b)")
                    if t == 3:
                        nc.scalar.copy(dst, tp)
                    else:
                        nc.vector.tensor_copy(dst, tp)
                # next chunk's DMA right after this chunk's transposes:
                # its buffer-release wait fires immediately, keeping chunk
                # completions progressive without head-blocking the FIFO
                if j + 3 < CJ:
                    emit_ctx_dma(j + 3)
                dots = dt_ps.tile([2, 512], f32, tag="dots")
                for jh in range(JH):
                    nc.tensor.matmul(
                        dots, w2[:, jh * 2:jh * 2 + 2],
                        ctxT[:, jh, :],
                        start=(jh == 0), stop=(jh == JH - 1))
                scgc = rowp.tile([2, 512], f32, tag="scgc")
                nc.scalar.copy(scgc, dots)

                # gc broadcast + 4 sigmoids (bias gh[gi]) -> bf16 sig tiles
                gc_b = bp.tile([P, 512], f32, tag="gc_b")
                nc.gpsimd.partition_broadcast(gc_b, scgc[0:1, :])

                # e row: DMA-hop sc to partition 0, DMA-accumulate the mask
                # row onto it, one Exp (bf16 out) with fused f32 partial-Z
                scr = rowp.tile([1, 512], f32, tag="scr")
                nc.gpsimd.dma_start(out=scr, in_=scgc[1:2, :])
                nc.gpsimd.dma_start(out=scr, in_=madd_d[0:1, ts(j, 512)],
                                    accum_op=mybir.AluOpType.add)
                e_row = rowp.tile([1, 512], bf16, tag="e_row")
                nc.scalar.activation(e_row, scr,
                                     mybir.ActivationFunctionType.Exp,
                                     accum_out=z_row[0:1, j:j + 1])
                e_b = bp.tile([P, 512], bf16, tag="e_b")
                nc.gpsimd.partition_broadcast(e_b, e_row)

                for gi in range(NGT):
                    sig_t = sigp.tile([P, 512], bf16, tag="sig_t")
                    nc.scalar.activation(
                        sig_t, gc_b,
                        mybir.ActivationFunctionType.Sigmoid,
                        bias=gh[:, gi:gi + 1])
                    nc.vector.tensor_mul(q[gi][:, ts(j, 512)], sig_t, e_b)

            # ---- Z, 1/Z, final scale + 2MB row DMAs ----
            z1 = smp.tile([1, 1], f32)
            nc.vector.reduce_sum(z1, z_row, axis=mybir.AxisListType.X)
            rz = smp.tile([1, 1], f32)
            nc.vector.reciprocal(rz, z1)
            rz_col = smp.tile([P, 1], f32)
            nc.gpsimd.partition_broadcast(rz_col, rz)
            for gi in range(NGT):
                out_t = outp.tile([P, C], f32, tag="out_t")
                nc.vector.tensor_scalar(out=out_t, in0=q[gi],
                                        scalar1=rz_col[:, 0:1],
                                        scalar2=None,
                                        op0=mybir.AluOpType.mult)
                nc.sync.dma_start(out=out_d[ts(gi, P), :], in_=out_t)

    nc.compile()
    return nc


def _get_nc():
    if "nc" not in _cache:
        _cache["nc"] = _build()
    return _cache["nc"]


def make_w3(w_attn, w_gate):
    # rows: (wg_c, wa_c, wg_h) — gc weight first so gc lands on partition 0
    return np.ascontiguousarray(
        np.stack([w_gate[H:], w_attn[H:], w_gate[:H]], axis=0),
        dtype=np.float32)


def make_in_maps(hidden_states, context_hidden, w_attn, w_gate, b_gate,
                 copy_mask):
    w3 = make_w3(w_attn, w_gate)
    bg = np.asarray(b_gate, dtype=np.float32).reshape(1, 1)
    in_maps = []
    for b in range(B):
        madd = np.where(np.asarray(copy_mask[b]) == 0, -1e5, 0.0)
        madd = madd.reshape(1, C).astype(np.float32)
        in_maps.append({
            "hid": np.ascontiguousarray(hidden_states[b], dtype=np.float32),
            "ctx": np.ascontiguousarray(context_hidden[b], dtype=np.float32),
            "madd": np.ascontiguousarray(madd),
            "w": w3,
            "bg": bg,
        })
    return in_maps


def kernel(hidden_states, context_hidden, encoder_output, w_attn, w_gate,
           b_gate, copy_mask):
    from concourse.bass_utils import run_bass_kernel_spmd

    nc = _get_nc()
    in_maps = make_in_maps(hidden_states, context_hidden, w_attn, w_gate,
                           b_gate, copy_mask)
    res = run_bass_kernel_spmd(nc, in_maps, core_ids=list(range(N_CORES)))
    return np.stack([res.results[b]["out"] for b in range(B)], axis=0)



# revision 14
# speedup vs baseline: 1.1824x; 1.1824x over previous
"""Trainium2 Bass kernel for nn_CopyMechanism (v2).

Math (per batch b, one NeuronCore per batch):
  out[g,c] = softmax_c(mask ? (score_h[g]+score_c[c]) : -inf)
             * sigmoid(gate_h[g]+gate_c[c]+b0)

softmax_c of (score_h[g]+score_c[c]) == softmax_c(score_c) (score_h constant
along c), so copy_probs is g-independent and w_attn[:H] drops out.
encoder_output is unused by the reference. Scores are O(1): exp needs no max
subtraction; masking is additive (sc - 1e5 -> exp underflows to 0).

v2 structure (fixes the v1 bottlenecks: 256 PE transposes ~70us, ACT table
thrash Exp<->Sigmoid ~18us, 2KB-descriptor SWDGE stream ~105us span):
  - ctx chunk DMA: partition p <- 4 CONSECUTIVE rows 4p..4p+3 ("(p ci) h"),
    giving 128 x 16KB-src descriptors per 2MB chunk (was 512 x 4KB), cast
    f32->bf16 in the SWDGE.
  - Transposes run on the f32 BITCAST view: one [128,128]-f32 transpose moves
    a [128,256]-bf16 block => 16 transposes/chunk instead of 32. The row
    permutation from the 4-consecutive-rows load is undone for free by a
    stride-4 PSUM out-AP on each transpose. Dot matmuls then read the bf16
    view with stride-2 (even/odd h) and parity-split weights, accumulating
    dots[2,512] (gc row 0, sc row 1) in f32 PSUM.
  - madd (mask) is added to the sc row only via a rank-1 f32r matmul
    (sel=[0,1] (x) madd_chunk), exact and ~0.2us.
  - gc is broadcast to all 128 partitions by a rank-1 f32r matmul
    (ones (x) gc_row) straight into PSUM for the ACT input.
  - ALL activations use the exp_and_others table set (ONE load total):
    sigmoid(x) is computed as tanh: q = (tanh(.5x)+1) * e * (0.5/Z).
    Per chunk: 4x Tanh [128,512] (bias = 0.5*(gh+b0) per partition, scale
    0.5, PSUM src) -> fp16, 1x Exp on the sc PSUM row -> e fp16 with fused
    f32 Z accumulation.
  - Per chunk DVE: (t+1) tensor_scalar fp16 4x-mode, then q = (t+1)*e_b
    tensor_tensor fp16; tail: q * (0.5/Z) tensor_scalar fp32-out 2x-mode,
    overlapped with 4x 2MB HWDGE output DMAs.
  - gh comes from the same f32-packed-transpose + matmul pipeline on hid
    (16 transposes + 8 N=512 matmuls), transposed to column layout [128,4]
    by four tiny rank-1 matmuls.
"""
import sys

if "/opt/trn_rl_repo" not in sys.path:
    sys.path.insert(0, "/opt/trn_rl_repo")

import numpy as np
from contextlib import ExitStack

B, G, C, H = 8, 512, 4096, 1024
N_CORES = 8
P = 128
CJ = C // 512          # 8 ctx chunks of 512 rows
NJ = 4                 # h2 (f32-pair) blocks of 128 per 1024-h
NGT = G // P           # 4 g-tiles of 128

_cache = {}


def _build():
    import concourse.bass as bass
    import concourse.tile as tile
    from concourse import bacc, mybir
    from concourse.masks import make_identity

    f32 = mybir.dt.float32
    bf16 = mybir.dt.bfloat16
    fp16 = mybir.dt.float16
    ts = bass.ts
    Act = mybir.ActivationFunctionType

    nc = bacc.Bacc("TRN2", target_bir_lowering=False, debug=False,
                   num_devices=N_CORES)
    hid_d = nc.dram_tensor("hid", [G, H], f32, kind="ExternalInput").ap()
    ctx_d = nc.dram_tensor("ctx", [C, H], f32, kind="ExternalInput").ap()
    madd_d = nc.dram_tensor("madd", [1, C], f32, kind="ExternalInput").ap()
    # gate weight wg_c, parity-split and replicated across 128 columns so the
    # dot matmul directly produces gc broadcast to all partitions
    wmatA_d = nc.dram_tensor("wmatA", [P, 8 * P], f32,
                             kind="ExternalInput").ap()
    # attn weight wa_c, parity-split [128, (J,pi)] -> sc row at partition 0
    wmatB_d = nc.dram_tensor("wmatB", [P, 8], f32, kind="ExternalInput").ap()
    # parity-split hid weights [128, (J,pi)] (wg_h)
    whid_d = nc.dram_tensor("whid", [P, 8], f32, kind="ExternalInput").ap()
    bg_d = nc.dram_tensor("bg", [1, 1], f32, kind="ExternalInput").ap()
    out_d = nc.dram_tensor("out", [G, C], f32, kind="ExternalOutput").ap()

    with tile.TileContext(nc) as tc:
        with ExitStack() as ctx:
            singles = ctx.enter_context(tc.tile_pool(name="singles", bufs=1))
            ctxp = ctx.enter_context(tc.tile_pool(name="ctxp", bufs=3))
            hidp = ctx.enter_context(tc.tile_pool(name="hidp", bufs=1))
            ctp = ctx.enter_context(tc.tile_pool(name="ctp", bufs=2))
            htp = ctx.enter_context(tc.tile_pool(name="htp", bufs=1))
            rowp = ctx.enter_context(tc.tile_pool(name="rowp", bufs=2))
            ebp = ctx.enter_context(tc.tile_pool(name="ebp", bufs=2))
            tp_ = ctx.enter_context(tc.tile_pool(name="tp_", bufs=3))
            t2p = ctx.enter_context(tc.tile_pool(name="t2p", bufs=3))
            qp = ctx.enter_context(tc.tile_pool(name="qp", bufs=1))
            outp = ctx.enter_context(tc.tile_pool(name="outp", bufs=2))
            smp = ctx.enter_context(tc.tile_pool(name="smp", bufs=1))
            tp_ps = ctx.enter_context(
                tc.tile_pool(name="tp_ps", bufs=2, space="PSUM"))
            dt_ps = ctx.enter_context(
                tc.tile_pool(name="dt_ps", bufs=2, space="PSUM"))
            gc_ps_p = ctx.enter_context(
                tc.tile_pool(name="gc_ps", bufs=2, space="PSUM"))

            # ---- big input DMAs (SWDGE cast f32->bf16) ----
            # partition p <- 4 consecutive rows 4p..4p+3: 128 x 16KB-src
            # descriptors per chunk
            ctx4s = []

            def emit_ctx_dma(j):
                ctx4 = ctxp.tile([P, 4, H], bf16, tag="ctx4")
                nc.gpsimd.dma_start(
                    out=ctx4,
                    in_=ctx_d[j * 512:(j + 1) * 512, :].rearrange(
                        "(p ci) h -> p ci h", p=P))
                ctx4s.append(ctx4)

            # small cast-DMAs first on the SWDGE queue (~0.3MB total, so they
            # land before the big ctx stream without delaying it much)
            wmatA_b = singles.tile([P, 8 * P], bf16)
            nc.gpsimd.dma_start(out=wmatA_b, in_=wmatA_d)
            wmatB_b = singles.tile([P, 8], bf16)
            nc.gpsimd.dma_start(out=wmatB_b, in_=wmatB_d)
            whid_b = singles.tile([P, 8], bf16)
            nc.gpsimd.dma_start(out=whid_b, in_=whid_d)
            madd_b = singles.tile([1, C], bf16)
            nc.gpsimd.dma_start(out=madd_b, in_=madd_d)
            bg_col = singles.tile([P, 1], f32)
            nc.gpsimd.dma_start(
                out=bg_col,
                in_=bass.AP(tensor=bg_d.tensor, offset=bg_d.offset,
                            ap=[[0, P], [1, 1]]))

            emit_ctx_dma(0)
            hid4 = hidp.tile([P, 4, H], bf16, tag="hid4")
            nc.gpsimd.dma_start(
                out=hid4,
                in_=hid_d.rearrange("(p ci) h -> p ci h", p=P))
            emit_ctx_dma(1)

            ident_f = singles.tile([P, P], f32)
            make_identity(nc, ident_f)
            ones_f = singles.tile([1, P], f32)
            nc.gpsimd.memset(ones_f[:, :], 1.0)
            one_b = singles.tile([1, 1], bf16)
            nc.gpsimd.memset(one_b[:, :], 1.0)

            # ---- gh = 0.5*(hid @ wg_h + b0), column layout [128, NGT] ----
            hid_f = hid4[:, :, :].bitcast(f32)        # [P, 4, 512]
            hidT = htp.tile([P, NJ, 512], f32, tag="hidT")
            for J in range(NJ):
                tp = tp_ps.tile([P, 512], f32, tag="tps")
                tpv = tp[:, :].rearrange("p (c ci) -> p ci c", ci=4)
                for ci in range(4):
                    nc.tensor.transpose(tpv[:, ci, :],
                                        hid_f[:, ci, ts(J, P)], ident_f)
                if J % 2 == 0:
                    nc.vector.tensor_copy(hidT[:, J, :], tp)
                else:
                    nc.scalar.copy(hidT[:, J, :], tp)
            gh_ps = dt_ps.tile([1, 512], f32, tag="dots")
            for J in range(NJ):
                hv = hidT[:, J, :].bitcast(bf16).rearrange(
                    "p (c two) -> p two c", two=2)
                for pi in range(2):
                    nc.tensor.matmul(
                        gh_ps, whid_b[:, J * 2 + pi:J * 2 + pi + 1],
                        hv[:, pi, :],
                        start=(J == 0 and pi == 0),
                        stop=(J == NJ - 1 and pi == 1))
            ghp_sb = smp.tile([1, 512], f32)
            nc.vector.tensor_copy(ghp_sb, gh_ps)
            # row -> column via 4 tiny rank-1 matmuls: out[:,gi] = row.T @ [1]
            ghc_ps = gc_ps_p.tile([P, 512], f32, tag="gcb")
            for gi in range(NGT):
                nc.tensor.matmul(ghc_ps[:, gi:gi + 1],
                                 ghp_sb[0:1, ts(gi, P)], ones_f[0:1, 0:1],
                                 start=True, stop=True)
            ghh = smp.tile([P, NGT], f32)
            nc.vector.tensor_scalar(out=ghh, in0=ghc_ps[:, 0:NGT],
                                    scalar1=bg_col[:, 0:1], scalar2=0.5,
                                    op0=mybir.AluOpType.add,
                                    op1=mybir.AluOpType.mult)

            # ---- persistent tiles ----
            z_row = smp.tile([1, CJ], f32)
            q = [qp.tile([P, C], fp16, tag=f"q{gi}", name=f"q{gi}")
                 for gi in range(NGT)]

            # ---- per-chunk pipeline ----
            for j in range(CJ):
                ctx4 = ctx4s[j]
                ctx_f = ctx4[:, :, :].bitcast(f32)    # [P, 4, 512]
                ctxT = ctp.tile([P, NJ, 512], f32, tag="ctxT")
                for J in range(NJ):
                    tp = tp_ps.tile([P, 512], f32, tag="tps")
                    tpv = tp[:, :].rearrange("p (c ci) -> p ci c", ci=4)
                    for ci in range(4):
                        nc.tensor.transpose(tpv[:, ci, :],
                                            ctx_f[:, ci, ts(J, P)], ident_f)
                    if J % 2 == 0:
                        nc.vector.tensor_copy(ctxT[:, J, :], tp)
                    else:
                        nc.scalar.copy(ctxT[:, J, :], tp)
                if j + 2 < CJ:
                    emit_ctx_dma(j + 2)

                # gc pre-broadcast [128, 512] (wg replicated across lhsT
                # cols) and sc row [1, 512] at partition 0, interleaved
                # accumulations
                gc_ps = gc_ps_p.tile([P, 512], f32, tag="gcb")
                sc_ps = dt_ps.tile([1, 512], f32, tag="dots")
                for J in range(NJ):
                    cv = ctxT[:, J, :].bitcast(bf16).rearrange(
                        "p (c two) -> p two c", two=2)
                    for pi in range(2):
                        first = (J == 0 and pi == 0)
                        last = (J == NJ - 1 and pi == 1)
                        nc.tensor.matmul(
                            gc_ps, wmatA_b[:, ts(J * 2 + pi, P)],
                            cv[:, pi, :], start=first, stop=last)
                        nc.tensor.matmul(
                            sc_ps, wmatB_b[:, J * 2 + pi:J * 2 + pi + 1],
                            cv[:, pi, :], start=first, stop=False)
                nc.tensor.matmul(sc_ps, one_b, madd_b[0:1, ts(j, 512)],
                                 start=False, stop=True)

                e_row = rowp.tile([1, 512], fp16, tag="e_row")
                nc.scalar.activation(e_row, sc_ps, Act.Exp,
                                     accum_out=z_row[0:1, j:j + 1])
                e_b = ebp.tile([P, 512], fp16, tag="e_b")
                nc.gpsimd.partition_broadcast(e_b, e_row)

                for gi in range(NGT):
                    t = tp_.tile([P, 512], fp16, tag="t")
                    nc.scalar.activation(t, gc_ps, Act.Tanh,
                                         bias=ghh[:, gi:gi + 1], scale=0.5)
                    t2 = t2p.tile([P, 512], fp16, tag="t2")
                    nc.vector.tensor_scalar(out=t2, in0=t, scalar1=1.0,
                                            scalar2=None,
                                            op0=mybir.AluOpType.add)
                    nc.vector.tensor_mul(q[gi][:, ts(j, 512)], t2, e_b)

            # ---- tail: Z, 0.5/Z, final scale + 4x 2MB row DMAs ----
            z1 = smp.tile([1, 1], f32)
            nc.vector.reduce_sum(z1, z_row, axis=mybir.AxisListType.X)
            rz = smp.tile([1, 1], f32)
            nc.vector.reciprocal(rz, z1)
            rzh = smp.tile([1, 1], f32)
            nc.vector.tensor_scalar(out=rzh, in0=rz, scalar1=0.5,
                                    scalar2=None, op0=mybir.AluOpType.mult)
            rz_col = smp.tile([P, 1], f32)
            nc.gpsimd.partition_broadcast(rz_col, rzh)
            for gi in range(NGT):
                out_t = outp.tile([P, C], f32, tag="out_t")
                nc.vector.tensor_scalar(out=out_t, in0=q[gi],
                                        scalar1=rz_col[:, 0:1],
                                        scalar2=None,
                                        op0=mybir.AluOpType.mult)
                nc.sync.dma_start(out=out_d[ts(gi, P), :], in_=out_t)

    nc.compile()
    return nc


def _get_nc():
    if "nc" not in _cache:
        _cache["nc"] = _build()
    return _cache["nc"]


def _make_weights(w_attn, w_gate):
    w_attn = np.asarray(w_attn, dtype=np.float32)
    w_gate = np.asarray(w_gate, dtype=np.float32)
    wg_c, wa_c = w_gate[H:], w_attn[H:]
    wg_h = w_gate[:H]
    k = np.arange(P)
    wmatA = np.zeros((P, 8 * P), dtype=np.float32)
    wmatB = np.zeros((P, 8), dtype=np.float32)
    whid = np.zeros((P, 8), dtype=np.float32)
    for J in range(4):
        for pi in range(2):
            h = J * 256 + 2 * k + pi
            wmatA[:, (J * 2 + pi) * P:(J * 2 + pi + 1) * P] = \
                np.repeat(wg_c[h][:, None], P, axis=1)
            wmatB[:, J * 2 + pi] = wa_c[h]
            whid[:, J * 2 + pi] = wg_h[h]
    return wmatA, wmatB, whid


def make_in_maps(hidden_states, context_hidden, w_attn, w_gate, b_gate,
                 copy_mask):
    wmatA, wmatB, whid = _make_weights(w_attn, w_gate)
    bg = np.asarray(b_gate, dtype=np.float32).reshape(1, 1)
    in_maps = []
    for b in range(B):
        madd = np.where(np.asarray(copy_mask[b]) == 0, -1e5, 0.0)
        madd = madd.reshape(1, C).astype(np.float32)
        in_maps.append({
            "hid": np.ascontiguousarray(hidden_states[b], dtype=np.float32),
            "ctx": np.ascontiguousarray(context_hidden[b], dtype=np.float32),
            "madd": np.ascontiguousarray(madd),
            "wmatA": wmatA,
            "wmatB": wmatB,
            "whid": whid,
            "bg": bg,
        })
    return in_maps


def kernel(hidden_states, context_hidden, encoder_output, w_attn, w_gate,
           b_gate, copy_mask):
    from concourse.bass_utils import run_bass_kernel_spmd

    nc = _get_nc()
    in_maps = make_in_maps(hidden_states, context_hidden, w_attn, w_gate,
                           b_gate, copy_mask)
    res = run_bass_kernel_spmd(nc, in_maps, core_ids=list(range(N_CORES)))
    return np.stack([res.results[b]["out"] for b in range(B)], axis=0)


# revision 15
# speedup vs baseline: 1.3225x; 1.1185x over previous
"""Trainium2 Bass kernel for nn_CopyMechanism (v2).

Math (per batch b, one NeuronCore per batch):
  out[g,c] = softmax_c(mask ? (score_h[g]+score_c[c]) : -inf)
             * sigmoid(gate_h[g]+gate_c[c]+b0)

softmax_c of (score_h[g]+score_c[c]) == softmax_c(score_c) (score_h constant
along c), so copy_probs is g-independent and w_attn[:H] drops out.
encoder_output is unused by the reference. Scores are O(1): exp needs no max
subtraction; masking is additive (sc - 1e5 -> exp underflows to 0).

v2 structure (fixes the v1 bottlenecks: 256 PE transposes ~70us, ACT table
thrash Exp<->Sigmoid ~18us, 2KB-descriptor SWDGE stream ~105us span):
  - ctx chunk DMA: partition p <- 4 CONSECUTIVE rows 4p..4p+3 ("(p ci) h"),
    giving 128 x 16KB-src descriptors per 2MB chunk (was 512 x 4KB), cast
    f32->bf16 in the SWDGE.
  - Transposes run on the f32 BITCAST view: one [128,128]-f32 transpose moves
    a [128,256]-bf16 block => 16 transposes/chunk instead of 32. The row
    permutation from the 4-consecutive-rows load is undone for free by a
    stride-4 PSUM out-AP on each transpose. Dot matmuls then read the bf16
    view with stride-2 (even/odd h) and parity-split weights, accumulating
    dots[2,512] (gc row 0, sc row 1) in f32 PSUM.
  - madd (mask) is added to the sc row only via a rank-1 f32r matmul
    (sel=[0,1] (x) madd_chunk), exact and ~0.2us.
  - gc is broadcast to all 128 partitions by a rank-1 f32r matmul
    (ones (x) gc_row) straight into PSUM for the ACT input.
  - ALL activations use the exp_and_others table set (ONE load total):
    sigmoid(x) is computed as tanh: q = (tanh(.5x)+1) * e * (0.5/Z).
    Per chunk: 4x Tanh [128,512] (bias = 0.5*(gh+b0) per partition, scale
    0.5, PSUM src) -> fp16, 1x Exp on the sc PSUM row -> e fp16 with fused
    f32 Z accumulation.
  - Per chunk DVE: (t+1) tensor_scalar fp16 4x-mode, then q = (t+1)*e_b
    tensor_tensor fp16; tail: q * (0.5/Z) tensor_scalar fp32-out 2x-mode,
    overlapped with 4x 2MB HWDGE output DMAs.
  - gh comes from the same f32-packed-transpose + matmul pipeline on hid
    (16 transposes + 8 N=512 matmuls), transposed to column layout [128,4]
    by four tiny rank-1 matmuls.
"""
import sys

if "/opt/trn_rl_repo" not in sys.path:
    sys.path.insert(0, "/opt/trn_rl_repo")

import numpy as np
from contextlib import ExitStack

B, G, C, H = 8, 512, 4096, 1024
N_CORES = 8
P = 128
CJ = C // 512          # 8 ctx chunks of 512 rows
NJ = 4                 # h2 (f32-pair) blocks of 128 per 1024-h
NGT = G // P           # 4 g-tiles of 128

_cache = {}


def _build():
    import concourse.bass as bass
    import concourse.tile as tile
    from concourse import bacc, mybir
    from concourse.masks import make_identity

    f32 = mybir.dt.float32
    bf16 = mybir.dt.bfloat16
    fp16 = mybir.dt.float16
    ts = bass.ts
    Act = mybir.ActivationFunctionType

    nc = bacc.Bacc("TRN2", target_bir_lowering=False, debug=False,
                   num_devices=N_CORES)
    hid_d = nc.dram_tensor("hid", [G, H], f32, kind="ExternalInput").ap()
    ctx_d = nc.dram_tensor("ctx", [C, H], f32, kind="ExternalInput").ap()
    madd_d = nc.dram_tensor("madd", [1, C], f32, kind="ExternalInput").ap()
    # gate weight wg_c, parity-split and replicated across 128 columns so the
    # dot matmul directly produces gc broadcast to all partitions
    wmatA_d = nc.dram_tensor("wmatA", [P, 8 * P], f32,
                             kind="ExternalInput").ap()
    # attn weight wa_c, parity-split [128, (J,pi)] -> sc row at partition 0
    wmatB_d = nc.dram_tensor("wmatB", [P, 8], f32, kind="ExternalInput").ap()
    # parity-split hid weights [128, (J,pi)] (wg_h)
    whid_d = nc.dram_tensor("whid", [P, 8], f32, kind="ExternalInput").ap()
    bg_d = nc.dram_tensor("bg", [1, 1], f32, kind="ExternalInput").ap()
    out_d = nc.dram_tensor("out", [G, C], f32, kind="ExternalOutput").ap()

    with tile.TileContext(nc) as tc:
        with ExitStack() as ctx:
            singles = ctx.enter_context(tc.tile_pool(name="singles", bufs=1))
            ctxp = ctx.enter_context(tc.tile_pool(name="ctxp", bufs=3))
            hidp = ctx.enter_context(tc.tile_pool(name="hidp", bufs=1))
            ctp = ctx.enter_context(tc.tile_pool(name="ctp", bufs=2))
            htp = ctx.enter_context(tc.tile_pool(name="htp", bufs=1))
            rowp = ctx.enter_context(tc.tile_pool(name="rowp", bufs=2))
            ebp = ctx.enter_context(tc.tile_pool(name="ebp", bufs=2))
            tp_ = ctx.enter_context(tc.tile_pool(name="tp_", bufs=3))
            t2p = ctx.enter_context(tc.tile_pool(name="t2p", bufs=3))
            qp = ctx.enter_context(tc.tile_pool(name="qp", bufs=1))
            outp = ctx.enter_context(tc.tile_pool(name="outp", bufs=2))
            smp = ctx.enter_context(tc.tile_pool(name="smp", bufs=1))
            tp_ps = ctx.enter_context(
                tc.tile_pool(name="tp_ps", bufs=2, space="PSUM"))
            dt_ps = ctx.enter_context(
                tc.tile_pool(name="dt_ps", bufs=2, space="PSUM"))
            gc_ps_p = ctx.enter_context(
                tc.tile_pool(name="gc_ps", bufs=2, space="PSUM"))

            # ---- big input DMAs (SWDGE cast f32->bf16) ----
            # partition p <- 4 consecutive rows 4p..4p+3: 128 x 16KB-src
            # descriptors per chunk
            ctx4s = []

            def emit_ctx_dma(j):
                ctx4 = ctxp.tile([P, 4, H], bf16, tag="ctx4")
                nc.gpsimd.dma_start(
                    out=ctx4,
                    in_=ctx_d[j * 512:(j + 1) * 512, :].rearrange(
                        "(p ci) h -> p ci h", p=P))
                ctx4s.append(ctx4)

            # small cast-DMAs first on the SWDGE queue (~0.3MB total, so they
            # land before the big ctx stream without delaying it much)
            wmatA_b = singles.tile([P, 8 * P], bf16)
            nc.gpsimd.dma_start(out=wmatA_b, in_=wmatA_d)
            wmatB_b = singles.tile([P, 8], bf16)
            nc.gpsimd.dma_start(out=wmatB_b, in_=wmatB_d)
            whid_b = singles.tile([P, 8], bf16)
            nc.gpsimd.dma_start(out=whid_b, in_=whid_d)
            madd_b = singles.tile([1, C], bf16)
            nc.gpsimd.dma_start(out=madd_b, in_=madd_d)
            bg_col = singles.tile([P, 1], f32)
            nc.gpsimd.dma_start(
                out=bg_col,
                in_=bass.AP(tensor=bg_d.tensor, offset=bg_d.offset,
                            ap=[[0, P], [1, 1]]))

            emit_ctx_dma(0)
            hid4 = hidp.tile([P, 4, H], bf16, tag="hid4")
            nc.gpsimd.dma_start(
                out=hid4,
                in_=hid_d.rearrange("(p ci) h -> p ci h", p=P))
            emit_ctx_dma(1)

            ident_f = singles.tile([P, P], f32)
            make_identity(nc, ident_f)
            ones_f = singles.tile([1, P], f32)
            nc.gpsimd.memset(ones_f[:, :], 1.0)
            one_b = singles.tile([1, 1], bf16)
            nc.gpsimd.memset(one_b[:, :], 1.0)

            # ---- persistent tiles ----
            z_row = smp.tile([1, CJ], f32)
            q = [qp.tile([P, C], fp16, tag=f"q{gi}", name=f"q{gi}")
                 for gi in range(NGT)]
            ghh = smp.tile([P, NGT], f32)

            def producer(j):
                """chunk j: transposes -> evac -> dots -> exp -> e_b DRE"""
                if j + 2 < CJ:
                    emit_ctx_dma(j + 2)
                ctx4 = ctx4s[j]
                ctx_f = ctx4[:, :, :].bitcast(f32)    # [P, 4, 512]
                ctxT = ctp.tile([P, NJ, 512], f32, tag="ctxT")
                for J in range(NJ):
                    tp = tp_ps.tile([P, 512], f32, tag="tps")
                    tpv = tp[:, :].rearrange("p (c ci) -> p ci c", ci=4)
                    for ci in range(4):
                        nc.tensor.transpose(tpv[:, ci, :],
                                            ctx_f[:, ci, ts(J, P)], ident_f)
                    if J % 2 == 0:
                        nc.vector.tensor_copy(ctxT[:, J, :], tp)
                    else:
                        nc.scalar.copy(ctxT[:, J, :], tp)

                gc_ps = gc_ps_p.tile([P, 512], f32, tag="gcb")
                sc_ps = dt_ps.tile([1, 512], f32, tag="dots")
                for J in range(NJ):
                    cv = ctxT[:, J, :].bitcast(bf16).rearrange(
                        "p (c two) -> p two c", two=2)
                    for pi in range(2):
                        first = (J == 0 and pi == 0)
                        last = (J == NJ - 1 and pi == 1)
                        nc.tensor.matmul(
                            gc_ps, wmatA_b[:, ts(J * 2 + pi, P)],
                            cv[:, pi, :], start=first, stop=last)
                        nc.tensor.matmul(
                            sc_ps, wmatB_b[:, J * 2 + pi:J * 2 + pi + 1],
                            cv[:, pi, :], start=first, stop=False)
                nc.tensor.matmul(sc_ps, one_b, madd_b[0:1, ts(j, 512)],
                                 start=False, stop=True)

                e_row = rowp.tile([1, 512], fp16, tag="e_row")
                nc.scalar.activation(e_row, sc_ps, Act.Exp,
                                     accum_out=z_row[0:1, j:j + 1])
                e_b = ebp.tile([P, 512], fp16, tag="e_b")
                nc.gpsimd.partition_broadcast(e_b, e_row)
                return gc_ps, e_b

            def consumer(j, gc_ps, e_b):
                """chunk j: 4x tanh -> (t+1) -> q = (t+1)*e_b"""
                for gi in range(NGT):
                    t = tp_.tile([P, 512], fp16, tag="t")
                    nc.scalar.activation(t, gc_ps, Act.Tanh,
                                         bias=ghh[:, gi:gi + 1], scale=0.5)
                    t2 = t2p.tile([P, 512], fp16, tag="t2")
                    nc.vector.tensor_scalar(out=t2, in0=t, scalar1=1.0,
                                            scalar2=None,
                                            op0=mybir.AluOpType.add)
                    nc.vector.tensor_mul(q[gi][:, ts(j, 512)], t2, e_b)

            def hid_prelude():
                # gh = 0.5*(hid @ wg_h + b0), column layout [128, NGT]
                hid_f = hid4[:, :, :].bitcast(f32)    # [P, 4, 512]
                hidT = htp.tile([P, NJ, 512], f32, tag="hidT")
                for J in range(NJ):
                    tp = tp_ps.tile([P, 512], f32, tag="tps")
                    tpv = tp[:, :].rearrange("p (c ci) -> p ci c", ci=4)
                    for ci in range(4):
                        nc.tensor.transpose(tpv[:, ci, :],
                                            hid_f[:, ci, ts(J, P)], ident_f)
                    if J % 2 == 0:
                        nc.vector.tensor_copy(hidT[:, J, :], tp)
                    else:
                        nc.scalar.copy(hidT[:, J, :], tp)
                gh_ps = dt_ps.tile([1, 512], f32, tag="dots")
                for J in range(NJ):
                    hv = hidT[:, J, :].bitcast(bf16).rearrange(
                        "p (c two) -> p two c", two=2)
                    for pi in range(2):
                        nc.tensor.matmul(
                            gh_ps, whid_b[:, J * 2 + pi:J * 2 + pi + 1],
                            hv[:, pi, :],
                            start=(J == 0 and pi == 0),
                            stop=(J == NJ - 1 and pi == 1))
                ghp_sb = smp.tile([1, 512], f32)
                nc.vector.tensor_copy(ghp_sb, gh_ps)
                # row -> column via 4 tiny rank-1 matmuls
                ghc_ps = gc_ps_p.tile([P, 512], f32, tag="gcb")
                for gi in range(NGT):
                    nc.tensor.matmul(ghc_ps[:, gi:gi + 1],
                                     ghp_sb[0:1, ts(gi, P)], ones_f[0:1, 0:1],
                                     start=True, stop=True)
                nc.vector.tensor_scalar(out=ghh, in0=ghc_ps[:, 0:NGT],
                                        scalar1=bg_col[:, 0:1], scalar2=0.5,
                                        op0=mybir.AluOpType.add,
                                        op1=mybir.AluOpType.mult)

            # ---- software-pipelined chunk loop (consumer lags one chunk) ----
            prev = producer(0)
            hid_prelude()
            for j in range(1, CJ):
                cur = producer(j)
                consumer(j - 1, *prev)
                prev = cur
            consumer(CJ - 1, *prev)

            # ---- tail: Z, 0.5/Z, final scale + 4x 2MB row DMAs ----
            z1 = smp.tile([1, 1], f32)
            nc.vector.reduce_sum(z1, z_row, axis=mybir.AxisListType.X)
            rz = smp.tile([1, 1], f32)
            nc.vector.reciprocal(rz, z1)
            rzh = smp.tile([1, 1], f32)
            nc.vector.tensor_scalar(out=rzh, in0=rz, scalar1=0.5,
                                    scalar2=None, op0=mybir.AluOpType.mult)
            rz_col = smp.tile([P, 1], f32)
            nc.gpsimd.partition_broadcast(rz_col, rzh)
            for gi in range(NGT):
                out_t = outp.tile([P, C], f32, tag="out_t")
                nc.vector.tensor_scalar(out=out_t, in0=q[gi],
                                        scalar1=rz_col[:, 0:1],
                                        scalar2=None,
                                        op0=mybir.AluOpType.mult)
                nc.sync.dma_start(out=out_d[ts(gi, P), :], in_=out_t)

    nc.compile()
    return nc


def _get_nc():
    if "nc" not in _cache:
        _cache["nc"] = _build()
    return _cache["nc"]


def _make_weights(w_attn, w_gate):
    w_attn = np.asarray(w_attn, dtype=np.float32)
    w_gate = np.asarray(w_gate, dtype=np.float32)
    wg_c, wa_c = w_gate[H:], w_attn[H:]
    wg_h = w_gate[:H]
    k = np.arange(P)
    wmatA = np.zeros((P, 8 * P), dtype=np.float32)
    wmatB = np.zeros((P, 8), dtype=np.float32)
    whid = np.zeros((P, 8), dtype=np.float32)
    for J in range(4):
        for pi in range(2):
            h = J * 256 + 2 * k + pi
            wmatA[:, (J * 2 + pi) * P:(J * 2 + pi + 1) * P] = \
                np.repeat(wg_c[h][:, None], P, axis=1)
            wmatB[:, J * 2 + pi] = wa_c[h]
            whid[:, J * 2 + pi] = wg_h[h]
    return wmatA, wmatB, whid


def make_in_maps(hidden_states, context_hidden, w_attn, w_gate, b_gate,
                 copy_mask):
    wmatA, wmatB, whid = _make_weights(w_attn, w_gate)
    bg = np.asarray(b_gate, dtype=np.float32).reshape(1, 1)
    in_maps = []
    for b in range(B):
        madd = np.where(np.asarray(copy_mask[b]) == 0, -1e5, 0.0)
        madd = madd.reshape(1, C).astype(np.float32)
        in_maps.append({
            "hid": np.ascontiguousarray(hidden_states[b], dtype=np.float32),
            "ctx": np.ascontiguousarray(context_hidden[b], dtype=np.float32),
            "madd": np.ascontiguousarray(madd),
            "wmatA": wmatA,
            "wmatB": wmatB,
            "whid": whid,
            "bg": bg,
        })
    return in_maps


def kernel(hidden_states, context_hidden, encoder_output, w_attn, w_gate,
           b_gate, copy_mask):
    from concourse.bass_utils import run_bass_kernel_spmd

    nc = _get_nc()
    in_maps = make_in_maps(hidden_states, context_hidden, w_attn, w_gate,
                           b_gate, copy_mask)
    res = run_bass_kernel_spmd(nc, in_maps, core_ids=list(range(N_CORES)))
    return np.stack([res.results[b]["out"] for b in range(B)], axis=0)
